# revision 3
# baseline (speedup 1.0000x reference)
"""MixConv depthwise conv (3x3/5x5/7x7 over 64-channel groups) on 8 NeuronCores.

Per core: 24 channels (8 of each kernel size). Split by measured engine costs:
  - PE (TensorEngine): 8 k=7 + 6 k=5 channels as banded-Toeplitz matmuls
    (k dx-passes, each folding all k dy-taps into a [112,112] band, W-shifts
    as free-dim offsets into a padded SBUF tile, 8 PSUM banks of 4 images).
  - DVE (VectorEngine): 8 k=3 + 2 k=5 channels as explicit taps, layout
    [128 partitions = (image, 28-row H-block)], free dim = halo'd patch.
    Each tap = tensor_scalar_mul (294ns, 4x mode) + tensor_add (875ns, 4x on
    this silicon) ping-ponging bf16 accumulators. An even/odd column-shifted
    pair (x_e, x_o) keeps every tap read 4-byte aligned.
  - ACT (ScalarEngine): all PSUM drains (per-bank strided copies) and the
    on-chip x_o shift-copies (removes the second HBM read of xd).

Everything rides in bf16 (PSUM accumulates fp32). Balance point: PE ~134us,
DVE ~134us, ACT ~70us, DMA ~123us aggregate per core.
"""

import numpy as np
import ml_dtypes

import concourse.bacc as bacc
import concourse.mybir as mybir
import concourse.tile as tile
from concourse.bass_utils import run_bass_kernel_spmd

BF16 = ml_dtypes.bfloat16

# Problem constants (hardcoded per contract)
N_IMGS = 32
H = W = 112
GROUP_KS = (3, 5, 7)
GROUP_SIZE = 64          # channels per group
N_CORES = 8
CH_PER_GROUP_PER_CORE = GROUP_SIZE // N_CORES   # 8

# --- PE-side layout ---------------------------------------------------------
RW = {7: W + 6, 5: W + 4}          # per-image region width in padded tile
F_MM = {k: 4 * RW[k] for k in RW}  # matmul free dim (4 images/chunk)
XC = {k: N_IMGS * RW[k] + 8 for k in RW}   # +8 slack for last-chunk over-read
XC_MAX = XC[7]                      # 3784
OCOLS = N_IMGS * W
N_CHUNK = N_IMGS // 4               # 8 chunks = 8 PSUM banks

# --- DVE-side layout --------------------------------------------------------
HB = 28                             # rows per H-block (4 blocks x 32 imgs = 128)
RH = {3: HB + 2, 5: HB + 4}         # stored rows per partition (halo)
RWP = {3: W + 2, 5: W + 4}          # stored cols per row (pad cols)
XDF = {k: RH[k] * RWP[k] + 4 for k in (3, 5)}   # +4 slack for shifted reads
ODF = HB * W                        # 3136 out elems per partition
TAPS = {k: [(dy, dx) for dy in range(-(k // 2), k // 2 + 1)
            for dx in range(-(k // 2), k // 2 + 1)] for k in (3, 5)}

# --- channel split & order --------------------------------------------------
# Per core 24 channels: j in 0..7, ks (7,5,3). k7 -> PE, k3 -> DVE,
# k5 -> DVE for j in K5_DVE_J else PE. Processing order interleaves PE and
# DVE channels so both engine streams stay fed.
K5_DVE_J = (2, 5)                   # which k5 channels go to DVE
CORE_SEQ = []                       # list of (k, j, 'pe'|'dve')
for j in range(CH_PER_GROUP_PER_CORE):
    for k in (7, 5, 3):
        if k == 7:
            CORE_SEQ.append((k, j, "pe"))
        elif k == 5:
            CORE_SEQ.append((k, j, "dve" if j in K5_DVE_J else "pe"))
        else:
            CORE_SEQ.append((k, j, "dve"))

PE_CH = [(k, j) for k, j, s in CORE_SEQ if s == "pe"]       # 14
DVE_CH = [(k, j) for k, j, s in CORE_SEQ if s == "dve"]     # 10
N_PE = len(PE_CH)
KS_PE = [k for k, _ in PE_CH]
TOFF = np.cumsum([0] + KS_PE).tolist()
N_TMAT = TOFF[-1]
N_DVE = len(DVE_CH)
KS_DVE = [k for k, _ in DVE_CH]
WOFF = np.cumsum([0] + [k * k for k in KS_DVE]).tolist()    # wdve col offsets
N_WD = WOFF[-1]

MM_MODE = f"bf16 pe={N_PE} dve={N_DVE} (k5_dve={len(K5_DVE_J)}) act-drain"

_BASS_CACHE = {}


def _build_bass(reps=1):
    bf = mybir.dt.bfloat16
    f32 = mybir.dt.float32
    nc = bacc.Bacc("TRN2", target_bir_lowering=False, debug=False)
    xp_d = nc.dram_tensor("xp", [N_PE, H, XC_MAX], bf, kind="ExternalInput")
    t_d = nc.dram_tensor("tmat", [N_TMAT * H, H], bf, kind="ExternalInput")
    y_d = nc.dram_tensor("y", [N_PE, H, OCOLS], bf, kind="ExternalOutput")
    xd_d = nc.dram_tensor("xd", [N_DVE, 128, XDF[5]], bf, kind="ExternalInput")
    w_d = nc.dram_tensor("wdve", [128, N_WD], f32, kind="ExternalInput")
    y2_d = nc.dram_tensor("y2", [N_DVE, 128, ODF], bf, kind="ExternalOutput")

    with tile.TileContext(nc) as tc:
        with (
            tc.tile_pool(name="xpool", bufs=4) as xpool,
            tc.tile_pool(name="tpool", bufs=1) as tpool,
            tc.tile_pool(name="opool", bufs=3) as opool,
            tc.tile_pool(name="dpool", bufs=2) as dpool,
            tc.tile_pool(name="spool", bufs=3) as spool,
            tc.tile_pool(name="pspool", bufs=8, space="PSUM") as pspool,
        ):
            # Toeplitz bank resident in SBUF, one per-channel slice DMA each.
            t_t = tpool.tile([H, N_TMAT * H], bf, tag="t", name="tmat_sb")
            for ci in range(N_PE):
                k = KS_PE[ci]
                nc.sync.dma_start(
                    t_t[:, TOFF[ci] * H: (TOFF[ci] + k) * H],
                    t_d[TOFF[ci] * H: (TOFF[ci] + k) * H].rearrange(
                        "(p d) m -> p (d m)", d=k
                    ),
                )
            w_t = tpool.tile([128, N_WD], f32, tag="w", name="wdve_sb")
            nc.sync.dma_start(w_t[:, :], w_d[:, :])

            def pe_channel(ci):
                k = KS_PE[ci]
                rw, f_mm, xc = RW[k], F_MM[k], XC[k]
                x_t = xpool.tile([H, XC_MAX], bf, tag="x", name=f"x{ci}")
                nc.sync.dma_start(x_t[:, :xc], xp_d[ci][:, :xc])
                out_t = opool.tile([H, OCOLS], bf, tag="o", name=f"o{ci}")
                pts = [
                    pspool.tile([H, F_MM[7]], f32, tag="ps", name=f"ps{ci}_{b}")
                    for b in range(N_CHUNK)
                ]
                # pass-major: one lhsT per dx, all 8 banks stream under it
                for dx in range(k):
                    lhsT = t_t[:, (TOFF[ci] + dx) * H: (TOFF[ci] + dx + 1) * H]
                    for b in range(N_CHUNK):
                        base = 4 * b * rw
                        nc.tensor.matmul(
                            pts[b][:, :f_mm],
                            lhsT=lhsT,
                            rhs=x_t[:, base + dx: base + dx + f_mm],
                            start=(dx == 0),
                            stop=(dx == k - 1),
                        )
                ov = out_t.rearrange("p (i w) -> p i w", i=N_IMGS)
                for b in range(N_CHUNK):
                    img0 = 4 * b
                    src = pts[b][:, :f_mm].rearrange(
                        "p (i r) -> p i r", i=4)[:, :, :W]
                    dst = ov[:, img0: img0 + 4, :]
                    nc.scalar.copy(dst, src)
                    if b == N_CHUNK // 2 - 1:
                        nc.sync.dma_start(
                            y_d[ci][:, : OCOLS // 2], out_t[:, : OCOLS // 2]
                        )
                nc.sync.dma_start(y_d[ci][:, OCOLS // 2:], out_t[:, OCOLS // 2:])

            def dve_channel(di):
                k = KS_DVE[di]
                rwp, xdf, pad = RWP[k], XDF[k], k // 2
                x_e = dpool.tile([128, XDF[5]], bf, tag="xe", name=f"xe{di}")
                nc.sync.dma_start(x_e[:, :xdf], xd_d[di][:, :xdf])
                x_o = dpool.tile([128, XDF[5]], bf, tag="xo", name=f"xo{di}")
                # on-chip shift-copy on ACT (saves a second HBM read of xd)
                nc.scalar.copy(x_o[:, : xdf - 1], x_e[:, 1:xdf])
                accs = [
                    spool.tile([128, ODF], bf, tag=f"a{j}", name=f"acc{j}_{di}")
                    for j in range(2)
                ]

                def tap_ap(dy, dx):
                    off = (pad + dy) * rwp + (pad + dx)
                    src, o = (x_e, off) if off % 2 == 0 else (x_o, off - 1)
                    return src[:, o: o + HB * rwp].rearrange(
                        "p (r c) -> p r c", r=HB
                    )[:, :, :W]

                def wap(t):
                    return w_t[:, WOFF[di] + t: WOFF[di] + t + 1]

                # order taps even-offset first so x_o's ACT copy has time
                taps = TAPS[k]
                parity = lambda dy, dx: ((pad + dy) * rwp + (pad + dx)) % 2
                taps = sorted(taps, key=lambda t: parity(*t))

                a3 = [a.rearrange("p (r c) -> p r c", r=HB) for a in accs]
                dy0, dx0 = taps[0]
                nc.vector.tensor_scalar_mul(a3[0], tap_ap(dy0, dx0), wap(0))
                cur = 0
                for t in range(1, len(taps)):
                    dy, dx = taps[t]
                    nxt = 1 - cur
                    s_t = spool.tile([128, ODF], bf, tag="s", name=f"s{di}_{t}")
                    s3 = s_t.rearrange("p (r c) -> p r c", r=HB)
                    nc.vector.tensor_scalar_mul(s3, tap_ap(dy, dx), wap(t))
                    nc.vector.tensor_add(a3[nxt], a3[cur], s3)
                    cur = nxt
                nc.sync.dma_start(y2_d[di], accs[cur][:, :])

            def body():
                pe_i = dve_i = 0
                for _, _, side in CORE_SEQ:
                    if side == "dve":
                        dve_channel(dve_i)
                        dve_i += 1
                    else:
                        pe_channel(pe_i)
                        pe_i += 1

            if reps == 1:
                body()
            else:
                with tc.For_i(0, reps, 1):
                    body()
    nc.compile()
    return nc


def _get_bass(reps=1):
    if reps not in _BASS_CACHE:
        _BASS_CACHE[reps] = _build_bass(reps)
    return _BASS_CACHE[reps]


def _build_toeplitz(w, k):
    """w: [C, 1, k, k] -> T: [C, k, H, H], T[c,dx,hin,hout] = w[c,0,hin-hout+pad,dx]."""
    pad = (k - 1) // 2
    C = w.shape[0]
    T = np.zeros((C, k, H, H), np.float32)
    for dy in range(k):
        off = pad - dy  # hout = hin + off
        hin = np.arange(max(0, -off), H - max(0, off))
        T[:, :, hin, hin + off] = w[:, 0, dy, :][:, :, None]
    return T


def _gch(k, j, core):
    """Global channel id for (kernel size k, slot j) on `core`."""
    g = {3: 0, 5: 1, 7: 2}[k]
    return g * GROUP_SIZE + core * CH_PER_GROUP_PER_CORE + j


def _prepare_in_maps(x, w3, w5, w7):
    x = np.asarray(x, dtype=np.float32).astype(BF16)
    ws = {3: np.asarray(w3, np.float32), 5: np.asarray(w5, np.float32),
          7: np.asarray(w7, np.float32)}
    Ts = {k: _build_toeplitz(ws[k], k) for k in (5, 7)}

    in_maps = []
    for core in range(N_CORES):
        # staged x (PE): data at [i*RW + pad, i*RW + pad + W) per image
        xp = np.zeros((N_PE, H, XC_MAX), BF16)
        for ci, (k, j) in enumerate(PE_CH):
            gch = _gch(k, j, core)
            rw, pad = RW[k], (k - 1) // 2
            xv = xp[ci, :, : N_IMGS * rw].reshape(H, N_IMGS, rw)
            xv[:, :, pad: pad + W] = x[:, gch].transpose(1, 0, 2)

        # tmat blocks: per PE channel [hin, dx, hout] -> [(hin dx), hout]
        blocks = []
        for ci, (k, j) in enumerate(PE_CH):
            gch = _gch(k, j, core)
            Tc = Ts[k][gch % GROUP_SIZE]  # [dx, hin, hout]
            blocks.append(
                np.ascontiguousarray(Tc.transpose(1, 0, 2)).reshape(k * H, H)
            )
        tml = np.concatenate(blocks, axis=0)
        assert tml.shape[0] == N_TMAT * H

        # staged x (DVE): [N_DVE, 128, XDF[5]]; partition = img*4 + hblock
        xd = np.zeros((N_DVE, 128, XDF[5]), BF16)
        wd = np.zeros((128, N_WD), np.float32)
        for di, (k, j) in enumerate(DVE_CH):
            gch = _gch(k, j, core)
            rh, rwp, pad, xdf = RH[k], RWP[k], k // 2, XDF[k]
            xdv = xd[di, :, : rh * rwp].reshape(N_IMGS, 4, rh, rwp)
            pad_img = np.zeros((N_IMGS, H + 2 * pad, rwp), BF16)
            pad_img[:, pad: H + pad, pad: pad + W] = x[:, gch]
            for hb in range(4):
                xdv[:, hb] = pad_img[:, HB * hb: HB * hb + rh, :]
            # tap weights broadcast across partitions, sorted like the kernel
            taps = TAPS[k]
            parity = lambda dy, dx: ((pad + dy) * rwp + (pad + dx)) % 2
            taps = sorted(taps, key=lambda t: parity(*t))
            wk = ws[k][gch % GROUP_SIZE, 0]
            for t, (dy, dx) in enumerate(taps):
                wd[:, WOFF[di] + t] = wk[dy + pad, dx + pad]

        in_maps.append({
            "xp": xp, "tmat": tml.astype(BF16), "xd": xd,
            "wdve": np.ascontiguousarray(wd),
        })
    return in_maps


def _gather(results):
    out = np.empty((N_IMGS, GROUP_SIZE * len(GROUP_KS), H, W), np.float32)
    for core in range(N_CORES):
        y = np.asarray(results[core]["y"]).astype(np.float32)
        y = y.reshape(N_PE, H, N_IMGS, W)
        for ci, (k, j) in enumerate(PE_CH):
            out[:, _gch(k, j, core)] = y[ci].transpose(1, 0, 2)
        y2 = np.asarray(results[core]["y2"]).astype(np.float32)
        y2 = y2.reshape(N_DVE, N_IMGS, 4, HB, W)
        for di, (k, j) in enumerate(DVE_CH):
            out[:, _gch(k, j, core)] = y2[di].reshape(N_IMGS, H, W)
    return out


def run(x, w3, w5, w7, **spmd_kwargs):
    """Full run; returns (output, BassKernelResults) for profiling access."""
    nc = _get_bass()
    in_maps = _prepare_in_maps(x, w3, w5, w7)
    br = run_bass_kernel_spmd(nc, in_maps, core_ids=list(range(N_CORES)), **spmd_kwargs)
    return _gather(br.results), br


def kernel(x, w3, w5, w7):
    out, _ = run(x, w3, w5, w7)
    return out
